# revision 8
# baseline (speedup 1.0000x reference)
"""Causal self-attention with AdaLN, tensor-parallel over 8 TRN2 NeuronCores.

Sharding: heads (16) split across 8 cores (2 heads/core). Each core:
  - computes AdaLN(x) (replicated) fused with transpose to (d, seq) layout
  - computes its q/k/v head columns (qkv matmul, q pre-scaled by 1/sqrt(hd))
  - runs causal attention for its 2 heads (both batches)
  - computes a partial output projection (row-parallel w_proj slice)
Host sums the 8 partial (B*S, D) outputs.

Self-contained: hardcodes B=2, S=2048, D=2048, H=16, hd=128.
"""

import numpy as np

import concourse.bacc as bacc
import concourse.bass as bass
import concourse.mybir as mybir
import concourse.tile as tile
from concourse.bass_utils import run_bass_kernel_spmd
from concourse.masks import make_causal_mask, make_identity

FP = mybir.dt.float32
P = 128
B, S, D = 2, 2048, 2048
NH, HD = 16, 128
NCORES = 8
HPC = NH // NCORES          # heads per core = 2
ROWS = B * S                # 4096
DK = D // P                 # 16 d-chunks of 128
NQKV = 3 * HPC * HD         # 768 qkv out channels per core
EPS = 1e-6
GAMMA_SCALE = 0.1
SG = 512                    # seq-group width for phase 1/2
AluOp = mybir.AluOpType
Act = mybir.ActivationFunctionType


def build_nc() -> bass.Bass:
    nc = bacc.Bacc(trn_type="TRN2")

    x_d = nc.dram_tensor("x", (ROWS, D), FP, kind="ExternalInput")
    gamma_d = nc.dram_tensor("gamma", (ROWS, D), FP, kind="ExternalInput")
    beta_d = nc.dram_tensor("beta", (ROWS, D), FP, kind="ExternalInput")
    # (D, 768): columns = [q_h0, q_h1, k_h0, k_h1, v_h0, v_h1] * 128; q cols pre-scaled
    wqkvT_d = nc.dram_tensor("wqkvT", (D, NQKV), FP, kind="ExternalInput")
    # (256, D): w_proj[:, core_slice].T
    wpT_d = nc.dram_tensor("wpT", (HPC * HD, D), FP, kind="ExternalInput")
    out_d = nc.dram_tensor("out", (ROWS, D), FP, kind="ExternalOutput")

    with tile.TileContext(nc) as tc:
        with (
            tc.tile_pool(name="const", bufs=1) as const_pool,
            tc.tile_pool(name="dram", bufs=1, space="DRAM") as dram_pool,
        ):
            ident = const_pool.tile([P, P], FP, name="ident")
            make_identity(nc, ident)
            cmask = const_pool.tile([P, P], FP, name="cmask")
            make_causal_mask(nc, cmask, mask_val=-1e30)
            epst = const_pool.tile([P, 1], FP, name="epst")
            nc.vector.memset(epst, EPS)
            neg10 = const_pool.tile([P, 1], FP, name="neg10")
            nc.vector.memset(neg10, -10.0)

            # DRAM scratch (dep-tracked via pool)
            qT_d = dram_pool.tile([HPC, HD, ROWS], FP, name="qT_s")   # (2,128,4096)
            kT_d = dram_pool.tile([HPC, HD, ROWS], FP, name="kT_s")
            v_d = dram_pool.tile([HPC, ROWS, HD], FP, name="v_s")     # (2,4096,128)
            oT_d = dram_pool.tile([HPC * HD, ROWS], FP, name="oT_s")  # (256,4096)

            # ---------------- Phase 1+2: AdaLN -> transpose -> QKV ----------
            with (
                tc.tile_pool(name="w12", bufs=1) as w12,
                tc.tile_pool(name="p12", bufs=2) as p12,
                tc.tile_pool(name="ps12", bufs=8, space="PSUM") as ps12,
            ):
                wq_sb = w12.tile([P, DK, NQKV], FP, name="wq_sb")
                nc.sync.dma_start(
                    out=wq_sb, in_=wqkvT_d.rearrange("(o p) n -> p o n", p=P)
                )

                for sg in range(ROWS // SG):  # 8 groups of 512 rows
                    xT = p12.tile([P, DK, SG], FP, tag="xT", name=f"xT{sg}")
                    for t in range(SG // P):  # 4 row-tiles
                        r0 = sg * SG + t * P
                        xt = p12.tile([P, D], FP, tag="xt", name=f"xt{sg}_{t}")
                        gt = p12.tile([P, D], FP, tag="gt", name=f"gt{sg}_{t}")
                        bt = p12.tile([P, D], FP, tag="bt", name=f"bt{sg}_{t}")
                        nc.sync.dma_start(out=xt, in_=x_d[r0 : r0 + P, :])
                        nc.sync.dma_start(out=gt, in_=gamma_d[r0 : r0 + P, :])
                        nc.sync.dma_start(out=bt, in_=beta_d[r0 : r0 + P, :])

                        st = p12.tile([P, 4, 6], FP, tag="st", name=f"st{sg}_{t}")
                        for i in range(4):
                            nc.vector.bn_stats(
                                out=st[:, i, :], in_=xt[:, i * 512 : (i + 1) * 512]
                            )
                        mv = p12.tile([P, 2], FP, tag="mv", name=f"mv{sg}_{t}")
                        nc.vector.bn_aggr(out=mv, in_=st)
                        rstd = p12.tile([P, 1], FP, tag="rstd", name=f"rs{sg}_{t}")
                        nc.scalar.activation(
                            out=rstd, in_=mv[:, 1:2], func=Act.Sqrt,
                            bias=epst, scale=1.0,
                        )
                        nc.vector.reciprocal(out=rstd, in_=rstd)
                        # xn = (x - mean) * rstd
                        nc.vector.tensor_scalar(
                            out=xt, in0=xt,
                            scalar1=mv[:, 0:1], scalar2=rstd,
                            op0=AluOp.subtract, op1=AluOp.mult,
                        )
                        # g = tanh((gamma-1)/0.1) = tanh(10*gamma - 10)
                        nc.scalar.activation(
                            out=gt, in_=gt, func=Act.Tanh, bias=neg10, scale=10.0
                        )
                        # g = 1 + 0.1*g
                        nc.vector.tensor_scalar(
                            out=gt, in0=gt, scalar1=GAMMA_SCALE, scalar2=1.0,
                            op0=AluOp.mult, op1=AluOp.add,
                        )
                        # adaln = g*xn + beta  (into bt)
                        nc.vector.tensor_tensor(out=gt, in0=gt, in1=xt, op=AluOp.mult)
                        nc.vector.tensor_tensor(out=bt, in0=gt, in1=bt, op=AluOp.add)
                        # transpose 16 (128,128) blocks into xT[:, db, t*128:...]
                        for db in range(DK):
                            pst = ps12.tile([P, 512], FP, tag="ps", name="pst")
                            nc.tensor.transpose(
                                pst[:, :P], bt[:, db * P : (db + 1) * P], ident
                            )
                            nc.scalar.copy(
                                out=xT[:, db, t * P : (t + 1) * P], in_=pst[:, :P]
                            )

                    # qkv matmuls for this seq group: out chunk (128 ch, 512 seq)
                    for nb in range(NQKV // P):  # 6
                        pq = ps12.tile([P, 512], FP, tag="ps", name="pq")
                        for d in range(DK):
                            nc.tensor.matmul(
                                pq,
                                lhsT=wq_sb[:, d, nb * P : (nb + 1) * P],
                                rhs=xT[:, d, :],
                                start=(d == 0),
                                stop=(d == DK - 1),
                            )
                        h = nb % HPC
                        sec = nb // HPC  # 0=q, 1=k, 2=v
                        if sec < 2:
                            qs = p12.tile([P, 512], FP, tag="qs", name="qs")
                            nc.vector.tensor_copy(out=qs, in_=pq)
                            dst = qT_d if sec == 0 else kT_d
                            nc.sync.dma_start(
                                out=dst[h, :, sg * SG : (sg + 1) * SG], in_=qs
                            )
                        else:
                            vs = p12.tile([P, 512], FP, tag="vs", name="vs")
                            nc.vector.tensor_copy(out=vs, in_=pq)
                            for j in range(4):
                                psv = ps12.tile([P, 512], FP, tag="ps", name="psv")
                                nc.tensor.transpose(
                                    psv[:, :P], vs[:, j * P : (j + 1) * P], ident
                                )
                                vtb = p12.tile([P, P], FP, tag="vtb", name="vtb")
                                nc.vector.tensor_copy(out=vtb, in_=psv[:, :P])
                                nc.sync.dma_start(
                                    out=v_d[h, sg * SG + j * P : sg * SG + (j + 1) * P, :],
                                    in_=vtb,
                                )

            # ---------------- Phase 3: causal attention per (b, h) ----------
            with (
                tc.tile_pool(name="p3", bufs=2) as p3,
                tc.tile_pool(name="ps3", bufs=8, space="PSUM") as ps3,
            ):
                NQT = S // P      # 16 q tiles of 128 per (b,h)
                NQG = S // 512    # 4 q groups of 512
                for b in range(B):
                    for h in range(HPC):
                        qT_sb = p3.tile([P, S], FP, tag="qT", name=f"qTs{b}{h}")
                        kT_sb = p3.tile([P, S], FP, tag="kT", name=f"kTs{b}{h}")
                        V_sb = p3.tile([P, S // P, HD], FP, tag="V", name=f"Vs{b}{h}")
                        nc.sync.dma_start(
                            out=qT_sb, in_=qT_d[h, :, b * S : (b + 1) * S]
                        )
                        nc.sync.dma_start(
                            out=kT_sb, in_=kT_d[h, :, b * S : (b + 1) * S]
                        )
                        nc.sync.dma_start(
                            out=V_sb,
                            in_=v_d[h, b * S : (b + 1) * S, :].rearrange(
                                "(c p) f -> p c f", p=P
                            ),
                        )
                        outT_sb = p3.tile([P, S], FP, tag="outT", name=f"oTs{b}{h}")

                        for qg in range(NQG):
                            probsT = p3.tile(
                                [P, NQT, 512], FP, tag="probsT", name=f"pT{b}{h}{qg}"
                            )
                            # zero diagonal-region blocks not written below
                            for kcl in range(1, 4):
                                kc = qg * 4 + kcl
                                for qt in range(kcl):
                                    nc.gpsimd.memset(
                                        probsT[:, kc, qt * P : (qt + 1) * P], 0.0
                                    )
                            for qt in range(4):
                                qq = qg * 512 + qt * P      # local q start
                                nk_all = qq + P             # causal k extent
                                nkc = nk_all // P           # # of 128-blocks
                                nch = (nk_all + 511) // 512  # # of 512 psum chunks
                                pss_list = []
                                for c5 in range(nch):
                                    nk = min(512, nk_all - c5 * 512)
                                    pss = ps3.tile([P, 512], FP, tag="ps", name="pss")
                                    nc.tensor.matmul(
                                        pss[:, :nk],
                                        lhsT=qT_sb[:, qq : qq + P],
                                        rhs=kT_sb[:, c5 * 512 : c5 * 512 + nk],
                                        start=True,
                                        stop=True,
                                    )
                                    pss_list.append((pss, nk))
                                # causal mask on the diagonal 128 cols
                                ci, off = divmod(qq, 512)
                                nc.vector.tensor_tensor(
                                    out=pss_list[ci][0][:, off : off + P],
                                    in0=pss_list[ci][0][:, off : off + P],
                                    in1=cmask,
                                    op=AluOp.add,
                                )
                                # softmax stats
                                mx = p3.tile([P, 4], FP, tag="mx", name="mx")
                                for c5, (pss, nk) in enumerate(pss_list):
                                    nc.vector.tensor_reduce(
                                        out=mx[:, c5 : c5 + 1], in_=pss[:, :nk],
                                        axis=mybir.AxisListType.X, op=AluOp.max,
                                    )
                                m = p3.tile([P, 1], FP, tag="m", name="m")
                                nc.vector.tensor_reduce(
                                    out=m, in_=mx[:, :nch],
                                    axis=mybir.AxisListType.X, op=AluOp.max,
                                )
                                negm = p3.tile([P, 1], FP, tag="negm", name="negm")
                                nc.vector.tensor_scalar_mul(negm, m, -1.0)
                                probs = p3.tile([P, S], FP, tag="probs", name="probs")
                                sm = p3.tile([P, 4], FP, tag="sm", name="sm")
                                for c5, (pss, nk) in enumerate(pss_list):
                                    nc.scalar.activation(
                                        out=probs[:, c5 * 512 : c5 * 512 + nk],
                                        in_=pss[:, :nk],
                                        func=Act.Exp, bias=negm, scale=1.0,
                                        accum_out=sm[:, c5 : c5 + 1],
                                    )
                                ssum = p3.tile([P, 1], FP, tag="ssum", name="ssum")
                                nc.vector.tensor_reduce(
                                    out=ssum, in_=sm[:, :nch],
                                    axis=mybir.AxisListType.X, op=AluOp.add,
                                )
                                rec = p3.tile([P, 1], FP, tag="rec", name="rec")
                                nc.vector.reciprocal(out=rec, in_=ssum)
                                nc.vector.tensor_scalar(
                                    out=probs[:, :nk_all], in0=probs[:, :nk_all],
                                    scalar1=rec, scalar2=None, op0=AluOp.mult,
                                )
                                # transpose each 128-block into probsT
                                for kc in range(nkc):
                                    pst3 = ps3.tile([P, 512], FP, tag="ps", name="pst3")
                                    nc.tensor.transpose(
                                        pst3[:, :P], probs[:, kc * P : (kc + 1) * P], ident
                                    )
                                    nc.vector.tensor_copy(
                                        out=probsT[:, kc, qt * P : (qt + 1) * P],
                                        in_=pst3[:, :P],
                                    )
                            # PV for this q group
                            nkc_g = (qg + 1) * 4
                            po = ps3.tile([P, 512], FP, tag="ps", name="po")
                            for kc in range(nkc_g):
                                nc.tensor.matmul(
                                    po,
                                    lhsT=V_sb[:, kc, :],
                                    rhs=probsT[:, kc, :],
                                    start=(kc == 0),
                                    stop=(kc == nkc_g - 1),
                                )
                            nc.vector.tensor_copy(
                                out=outT_sb[:, qg * 512 : (qg + 1) * 512], in_=po
                            )
                        nc.sync.dma_start(
                            out=oT_d[h * P : (h + 1) * P, b * S : (b + 1) * S],
                            in_=outT_sb,
                        )

            # ---------------- Phase 4: partial output projection -------------
            with (
                tc.tile_pool(name="p4", bufs=3) as p4,
                tc.tile_pool(name="ps4", bufs=4, space="PSUM") as ps4,
            ):
                oT_sb = p4.tile([P, HPC, ROWS], FP, tag="oT", bufs=1, name="oT_sb")
                nc.sync.dma_start(
                    out=oT_sb, in_=oT_d[:].rearrange("(o p) q -> p o q", p=P)
                )
                wp_sb = p4.tile([P, HPC, D], FP, tag="wp", bufs=1, name="wp_sb")
                nc.sync.dma_start(
                    out=wp_sb, in_=wpT_d.rearrange("(o p) j -> p o j", p=P)
                )
                for qb in range(ROWS // P):  # 32
                    for jc in range(D // 512):  # 4
                        pp = ps4.tile([P, 512], FP, tag="ps", name="pp")
                        for i in range(HPC):
                            nc.tensor.matmul(
                                pp,
                                lhsT=oT_sb[:, i, qb * P : (qb + 1) * P],
                                rhs=wp_sb[:, i, jc * 512 : (jc + 1) * 512],
                                start=(i == 0),
                                stop=(i == HPC - 1),
                            )
                        osb = p4.tile([P, 512], FP, tag="os", name="osb")
                        nc.vector.tensor_copy(out=osb, in_=pp)
                        nc.sync.dma_start(
                            out=out_d[qb * P : (qb + 1) * P, jc * 512 : (jc + 1) * 512],
                            in_=osb,
                        )
    nc.finalize()
    return nc


_NC_CACHE: bass.Bass | None = None


def _get_nc() -> bass.Bass:
    global _NC_CACHE
    if _NC_CACHE is None:
        _NC_CACHE = build_nc()
    return _NC_CACHE


def _make_in_maps(x, gamma, beta, w_qkv, w_proj):
    x2 = np.ascontiguousarray(np.asarray(x, np.float32).reshape(ROWS, D))
    g2 = np.ascontiguousarray(np.asarray(gamma, np.float32).reshape(ROWS, D))
    b2 = np.ascontiguousarray(np.asarray(beta, np.float32).reshape(ROWS, D))
    w_qkv = np.asarray(w_qkv, np.float32)
    w_proj = np.asarray(w_proj, np.float32)
    scale = 1.0 / np.sqrt(HD)
    in_maps = []
    for c in range(NCORES):
        h0 = c * HPC
        rows = []
        for sec in range(3):  # q, k, v
            for hl in range(HPC):
                blk = w_qkv[sec * D + (h0 + hl) * HD : sec * D + (h0 + hl + 1) * HD, :]
                if sec == 0:
                    blk = blk * scale
                rows.append(blk)
        w_c = np.concatenate(rows, axis=0)  # (768, 2048)
        wqkvT = np.ascontiguousarray(w_c.T)  # (2048, 768)
        wpT = np.ascontiguousarray(
            w_proj[:, h0 * HD : (h0 + HPC) * HD].T
        )  # (256, 2048)
        in_maps.append(
            {"x": x2, "gamma": g2, "beta": b2, "wqkvT": wqkvT, "wpT": wpT}
        )
    return in_maps


def run_cores(x, gamma, beta, w_qkv, w_proj, trace=False, **kwargs):
    nc = _get_nc()
    in_maps = _make_in_maps(x, gamma, beta, w_qkv, w_proj)
    res = run_bass_kernel_spmd(
        nc, in_maps, list(range(NCORES)), trace=trace, **kwargs
    )
    partials = [res.results[c]["out"] for c in range(NCORES)]
    acc = np.zeros((ROWS, D), np.float64)
    for p_arr in partials:
        acc += p_arr.astype(np.float64)
    out = acc.astype(np.float32).reshape(B, S, D)
    return out, res


def kernel(x, gamma, beta, w_qkv, w_proj):
    out, _ = run_cores(x, gamma, beta, w_qkv, w_proj, trace=False)
    return out
